# revision 6
# baseline (speedup 1.0000x reference)
"""Trainium2 Bass kernel for nn_MeshNorms (gnn_message_passing).

The oracle's inputs are a regular 1025x1025 grid mesh: `faces` / `normmap`
are deterministic functions of the grid, so every gather in the reference is
a shifted-window (stencil) read.  The kernel verifies that structure on the
host (cheap numpy check) and runs a streaming stencil kernel on 8 cores:

  sharding: 2 batches x 4 row-slices; each core computes 256 output rows as
  2 chunks of 128 grid rows (partition dim = grid row).

  All device math is fp16 so every DVE tensor_tensor op engages the 2x_1P
  perf mode (16-bit dtype, unit stride, 4B-aligned, even width).  Shifted
  (odd-offset) operands are produced either by extra shifted DMA loads from
  DRAM (a0s/a1s) or by ACT-engine copies (m(j+1), q(j+1)) which run at 1x
  regardless of alignment.  Squares and rsqrt (raw InstActivation(Rsqrt),
  max rel err ~5e-5, bias=eps folded in) run on the ACT engine in parallel
  with the DVE stream.  tri1/tri2 face-normal planes are packed side by side
  ([... , 2*1028]) so one instruction covers both triangle families.

  The DVE instruction stream is software-pipelined: chunk n's tail
  (vertex-sum normalize + output) is emitted after chunk n+1's head so the
  ACT/DMA latencies hide behind useful DVE work.

Boundary handling: vertex columns are replicate-padded on the host (padded
cells produce exact zero normals); the row-1024 output and the per-core
b-halo row are computed on the host (tiny).

If the structure check fails, falls back to a numpy implementation.
"""

import numpy as np

GRID = 1025
NCELL = GRID - 1
V = GRID * GRID
F = 2 * NCELL * NCELL
B = 2
WP = GRID + 2              # 1027 padded vertex cols
WT = 1028                  # per-plane tile width (even)
W2 = 2 * WT                # paired tri1|tri2 width
WE = 1026                  # even valid width for face/sum ops
WO = GRID                  # 1025 output cols
CHUNK = 128
NCHUNK = 2
ROWS = CHUNK * NCHUNK      # 256 output rows per core
N_CORES = 8
EPS = 1e-6

_NC_CACHE = {}
TRACE = False
LAST_PERF = None


# ---------------------------------------------------------------- host math

def _grid_faces(n):
    idx = np.arange(n * n, dtype=np.int64).reshape(n, n)
    v00 = idx[:-1, :-1]; v01 = idx[:-1, 1:]
    v10 = idx[1:, :-1]; v11 = idx[1:, 1:]
    tri1 = np.stack([v00, v10, v01], axis=-1).reshape(-1, 3)
    tri2 = np.stack([v01, v10, v11], axis=-1).reshape(-1, 3)
    return np.concatenate([tri1, tri2], axis=0)


def _expected_normmap(n):
    nc = n - 1
    i, j = np.meshgrid(np.arange(n, dtype=np.int64),
                       np.arange(n, dtype=np.int64), indexing="ij")
    sent = np.int64(1) << 60

    def t1(ii, jj):
        valid = (ii >= 0) & (ii < nc) & (jj >= 0) & (jj < nc)
        return np.where(valid, ii * nc + jj, sent)

    def t2(ii, jj):
        valid = (ii >= 0) & (ii < nc) & (jj >= 0) & (jj < nc)
        return np.where(valid, nc * nc + ii * nc + jj, sent)

    cand = np.stack([t1(i - 1, j), t1(i, j - 1), t1(i, j),
                     t2(i - 1, j - 1), t2(i - 1, j), t2(i, j - 1)], axis=-1)
    cand.sort(axis=-1)
    cand = cand.reshape(n * n, 6)
    cand[cand == sent] = 2 * nc * nc
    return cand


def _is_grid_mesh(verts, faces, normmap):
    if verts.shape != (B, V, 3) or faces.shape != (F, 3) or normmap.shape != (V, 6):
        return False
    if not np.array_equal(faces, _grid_faces(GRID)):
        return False
    return np.array_equal(normmap, _expected_normmap(GRID))


def _fallback(verts, faces, normmap):
    verts = np.asarray(verts, np.float32)
    faces = np.asarray(faces)
    normmap = np.asarray(normmap)
    tri = verts[:, faces, :]
    v1 = tri[..., 0, :] - tri[..., 1, :]
    v2 = tri[..., 0, :] - tri[..., 2, :]
    cr = np.cross(v1, v2).astype(np.float32)
    fn = cr / np.linalg.norm(cr, axis=-1, keepdims=True)
    bb = fn.shape[0]
    fnp = np.concatenate([fn, np.zeros((bb, 1, 3), fn.dtype)], axis=1)
    vn = fnp[:, normmap, :].sum(axis=-2)
    vn = vn / np.linalg.norm(vn, axis=-1, keepdims=True)
    return vn.astype(np.float32)


def _cross3(u, v):
    return np.stack([u[1] * v[2] - u[2] * v[1],
                     u[2] * v[0] - u[0] * v[2],
                     u[0] * v[1] - u[1] * v[0]], 0).astype(np.float32)


def _normalize3(x, eps=np.float32(1e-12)):
    nsq = (x[0] * x[0] + x[1] * x[1]) + x[2] * x[2]
    s = np.sqrt(nsq + eps, dtype=np.float32)
    return (x * (np.float32(1.0) / s)).astype(np.float32)


def _host_face_row_b(gp, fr):
    """b(fr, j) = m(j+1) + p(j) + p(j+1) for one face row from the padded
    planar f32 grid gp [3, GRID, WP].  Returns [3, WO] float32."""
    WF = GRID + 1
    a0 = gp[:, fr:fr + 1, :]
    a1 = gp[:, fr + 1:fr + 2, :]
    er = a0 - a1
    ec = a0[:, :, :WF] - a0[:, :, 1:]
    dd = a0[:, :, 1:] - a1[:, :, :WF]
    m = _normalize3(_cross3(er[:, :, :WF], ec))
    p = _normalize3(_cross3(dd, er[:, :, 1:]))
    u = m[:, :, 1:] + p[:, :, :WO]
    bb = u + p[:, :, 1:]
    return bb[:, 0, :]


def _build_in_maps(verts):
    """Host prep: padded fp16 planar slabs per core + f32 halo rows.
    Returns (in_maps, gp32row_fn) where in_maps[core] = {vin, bh}."""
    verts = np.asarray(verts, np.float32)
    g = verts.reshape(B, GRID, GRID, 3)
    # full f32 padded planar grid (for host halo rows + last row)
    gp32 = np.empty((B, 3, GRID, WP), np.float32)
    gp32[:, :, :, 1:GRID + 1] = g.transpose(0, 3, 1, 2)
    gp32[:, :, :, 0] = gp32[:, :, :, 1]
    gp32[:, :, :, GRID + 1] = gp32[:, :, :, GRID]
    gp16 = gp32.astype(np.float16)

    in_maps = []
    for core in range(N_CORES):
        b, j = divmod(core, 4)
        r0 = j * ROWS
        slab = np.ascontiguousarray(
            gp16[b, :, r0:r0 + ROWS + 1, :].transpose(1, 0, 2))
        bh = np.zeros((1, 3, WE), np.float16)
        if j > 0:
            bh[0, :, :WO] = _host_face_row_b(gp32[b], r0 - 1).astype(np.float16)
        in_maps.append({"vin": slab, "bh": bh})
    return in_maps, gp32


# ------------------------------------------------------------- device build

def _act_rsqrt(nc, act, mybir, out, in_, bias):
    """Raw InstActivation(Rsqrt): out = rsqrt(in_ + bias). ~5e-5 max rel."""
    AF = mybir.ActivationFunctionType
    ins = [act.lower_ap(in_),
           mybir.ImmediateValue(dtype=mybir.dt.float32, value=float(bias)),
           mybir.ImmediateValue(dtype=mybir.dt.float32, value=1.0),
           mybir.ImmediateValue(dtype=mybir.dt.float32, value=0.0)]
    return act.add_instruction(mybir.InstActivation(
        name=nc.get_next_instruction_name(), func=AF.Rsqrt,
        ins=ins, outs=[act.lower_ap(out)]))


def _build_nc(repeat=1):
    from contextlib import ExitStack
    import concourse.bass as bass
    import concourse.mybir as mybir

    f16 = mybir.dt.float16
    AF = mybir.ActivationFunctionType

    nc = bass.Bass()
    vin = nc.dram_tensor("vin", [ROWS + 1, 3, WP], f16, kind="ExternalInput")
    bh = nc.dram_tensor("bh", [1, 3, WE], f16, kind="ExternalInput")
    out = nc.dram_tensor("out", [ROWS, 3, WO], f16, kind="ExternalOutput")

    N = NCHUNK * repeat

    # ---- precompute semaphore mark values by replaying emission order ----
    # DVE stream: head(0) mid(0) | head(n) tail(n-1) mid(n) ... | tail(N-1)
    vm = {}
    cnt = 0
    def _vrec(kind, n):
        nonlocal cnt
        cnt += 1
        vm[(kind, n)] = cnt
    for n in range(N):
        _vrec("AREL", n); _vrec("NN", n)          # head(n)
        if n > 0:
            _vrec("VN", n - 1); _vrec("VQB", n - 1); _vrec("OT", n - 1)
        _vrec("QB", n); _vrec("MM", n); _vrec("QQ", n); _vrec("BB", n)
    _vrec("VN", N - 1); _vrec("VQB", N - 1); _vrec("OT", N - 1)

    # ACT stream: SQ(0) | RS(n) MS(n) QS(n) [SQ(n+1)] VSQ(n) VRS(n) ...
    am = {}
    acnt = 0
    def _arec(kind, n):
        nonlocal acnt
        acnt += 1
        am[(kind, n)] = acnt
    _arec("SQ", 0)
    for n in range(N):
        _arec("RS", n); _arec("MS", n); _arec("QS", n)
        if n + 1 < N:
            _arec("SQ", n + 1)
        _arec("VSQ", n); _arec("VRS", n)

    with ExitStack() as ctx:
        sb = lambda shape, name: ctx.enter_context(
            nc.sbuf_tensor(name, shape, f16))
        A0 = [sb([CHUNK, 3, WT], "a0_0")]
        A1 = [sb([CHUNK, 3, WT], "a1_0")]
        A0S = [sb([CHUNK, 3, WT], "a0s_0")]
        A1S = [sb([CHUNK, 3, WT], "a1s_0")]
        E = sb([CHUNK, 3, W2], "E")      # er | dd
        Ft = sb([CHUNK, 3, W2], "F")     # ec | ers
        T1 = sb([CHUNK, 3, W2], "T1")
        T2 = sb([CHUNK, 3, W2], "T2")
        NNt = sb([CHUNK, 3, W2], "NN")   # n1 | n2
        QBt = sb([CHUNK, W2], "QB")
        RRt = sb([CHUNK, W2], "RR")
        MMt = sb([CHUNK, 3, W2], "MM")   # m | p
        MS = sb([CHUNK, 3, WE], "MS")    # m(j+1)
        QS = sb([CHUNK, 3, WE], "QS")    # q(j+1)
        QQ = sb([CHUNK, 3, WE], "QQ")    # m + p
        AA = sb([CHUNK, 3, WE], "AA")
        VN = sb([CHUNK, 3, WE], "VN")
        BBt = sb([CHUNK, 3, WE], "BB")
        BSH = sb([CHUNK, 3, WE], "BSH")
        VQB = sb([CHUNK, WE], "VQB")
        RV = sb([CHUNK, WE], "RV")
        OT = [sb([CHUNK, 3, WE], f"ot_{s}") for s in range(2)]

        # aliases into dead tiles (guarded by asem waits in head())
        SQ = T1            # ACT squares of NN        [3, W2]
        VSQ = T2           # ACT vertex squares ->    [3, 0:WE]

        sem_in = ctx.enter_context(nc.semaphore("sem_in"))
        sem_bsh = ctx.enter_context(nc.semaphore("sem_bsh"))
        sem_out = ctx.enter_context(nc.semaphore("sem_out"))
        vsem = ctx.enter_context(nc.semaphore("vsem"))
        asem = ctx.enter_context(nc.semaphore("asem"))
        block = ctx.enter_context(nc.Block())

        def rows(n):
            r = (n % NCHUNK) * CHUNK
            return r

        @block.vector
        def _(dve):
            def seg2(ap3, c, width=WE):
                """[128, 2, width] view pairing the tri1/tri2 segments of
                component c of a [CHUNK, 3, W2] tile."""
                return ap3[:, c, 0:W2].rearrange(
                    "p (s w) -> p s w", s=2)[:, :, 0:width]

            def head(n):
                a0, a1 = A0[0].ap(), A1[0].ap()
                a0s, a1s = A0S[0].ap(), A1S[0].ap()
                e, f = E.ap(), Ft.ap()
                dve.wait_ge(sem_in, 64 * (n + 1))
                dve.tensor_sub(e[:, :, 0:WT], a0, a1)                    # er
                dve.tensor_sub(e[:, :, WT:W2], a0s, a1)                  # dd
                dve.tensor_sub(f[:, :, 0:WT], a0, a0s)                   # ec
                dve.tensor_sub(f[:, :, WT:W2], a0s, a1s).then_inc(vsem, 1)  # ers
                # T1/T2 are also ACT's SQ(n-1) output and NNt is SQ(n-1)'s
                # input; VSQ(n-2) writes T2 — guard both aliases.
                if n >= 1:
                    dve.wait_ge(asem, am[("SQ", n - 1)])
                if n >= 2:
                    dve.wait_ge(asem, am[("VSQ", n - 2)])
                t1, t2, nn = T1.ap(), T2.ap(), NNt.ap()
                for c in range(3):
                    u1, u2 = (c + 1) % 3, (c + 2) % 3
                    dve.tensor_mul(seg2(t1, c), seg2(e, u1), seg2(f, u2))
                for c in range(3):
                    u1, u2 = (c + 1) % 3, (c + 2) % 3
                    dve.tensor_mul(seg2(t2, c), seg2(e, u2), seg2(f, u1))
                dve.tensor_sub(nn, t1, t2).then_inc(vsem, 1)             # NN

            def mid(n):
                qb, rr, nn, mm = QBt.ap(), RRt.ap(), NNt.ap(), MMt.ap()
                sq = SQ.ap()
                dve.wait_ge(asem, am[("SQ", n)])
                dve.tensor_add(qb, sq[:, 0, :], sq[:, 1, :])
                dve.tensor_add(qb, qb, sq[:, 2, :]).then_inc(vsem, 1)    # QB
                dve.wait_ge(asem, am[("RS", n)])
                ins = None
                for c in range(3):
                    ins = dve.tensor_mul(mm[:, c, :], nn[:, c, :], rr)
                ins.then_inc(vsem, 1)                                    # MM
                dve.tensor_add(QQ.ap(), mm[:, :, 0:WE],
                               mm[:, :, WT:WT + WE]).then_inc(vsem, 1)   # QQ
                dve.wait_ge(asem, am[("MS", n)])
                dve.tensor_add(AA.ap(), QQ.ap(), MS.ap())
                dve.wait_ge(asem, am[("QS", n)])
                if n >= 1:
                    dve.wait_ge(sem_bsh, 32 * n + 16)   # bsh0(n) landed
                dve.tensor_add(BBt.ap(), QS.ap(),
                               mm[:, :, WT:WT + WE]).then_inc(vsem, 1)   # BB

            def tail(n):
                vn = VN.ap()
                dve.wait_ge(sem_bsh, 32 * n + 32)
                dve.tensor_add(vn, AA.ap(), BSH.ap()).then_inc(vsem, 1)  # VN
                vsq = VSQ.ap()
                dve.wait_ge(asem, am[("VSQ", n)])
                dve.tensor_add(VQB.ap(), vsq[:, 0, 0:WE], vsq[:, 1, 0:WE])
                dve.tensor_add(VQB.ap(), VQB.ap(),
                               vsq[:, 2, 0:WE]).then_inc(vsem, 1)        # VQB
                dve.wait_ge(asem, am[("VRS", n)])
                if n >= 2:
                    dve.wait_ge(sem_out, 16 * (n - 1))
                ot = OT[n % 2].ap()
                ins = None
                for c in range(3):
                    ins = dve.tensor_mul(ot[:, c, :], vn[:, c, :], RV.ap())
                ins.then_inc(vsem, 1)                                    # OT

            for n in range(N):
                head(n)
                if n > 0:
                    tail(n - 1)
                mid(n)
            tail(N - 1)

        @block.scalar
        def _(act):
            def squares(n):
                act.wait_ge(vsem, vm[("NN", n)])
                ins = None
                for c in range(3):
                    ins = act.activation(SQ.ap()[:, c, :], NNt.ap()[:, c, :],
                                         AF.Square)
                ins.then_inc(asem, 1)                                    # SQ
            squares(0)
            for n in range(N):
                act.wait_ge(vsem, vm[("QB", n)])
                _act_rsqrt(nc, act, mybir, RRt.ap(), QBt.ap(), EPS) \
                    .then_inc(asem, 1)                                   # RS
                act.wait_ge(vsem, vm[("MM", n)])
                act.activation(MS.ap(), MMt.ap()[:, :, 1:1 + WE], AF.Copy) \
                    .then_inc(asem, 1)                                   # MS
                act.wait_ge(vsem, vm[("QQ", n)])
                act.activation(QS.ap()[:, :, 0:WE - 1],
                               QQ.ap()[:, :, 1:WE], AF.Copy) \
                    .then_inc(asem, 1)                                   # QS
                if n + 1 < N:
                    squares(n + 1)
                act.wait_ge(vsem, vm[("VN", n)])
                ins = None
                for c in range(3):
                    ins = act.activation(VSQ.ap()[:, c, 0:WE],
                                         VN.ap()[:, c, :], AF.Square)
                ins.then_inc(asem, 1)                                    # VSQ
                act.wait_ge(vsem, vm[("VQB", n)])
                _act_rsqrt(nc, act, mybir, RV.ap(), VQB.ap(), EPS) \
                    .then_inc(asem, 1)                                   # VRS

        @block.sync
        def _(sp):
            def loads(n):
                r = rows(n)
                sp.dma_start(A0[0].ap()[:, :, 0:WP], vin[r:r + CHUNK]) \
                  .then_inc(sem_in, 16)
                sp.dma_start(A1[0].ap()[:, :, 0:WP], vin[r + 1:r + CHUNK + 1]) \
                  .then_inc(sem_in, 16)
                sp.dma_start(A0S[0].ap()[:, :, 0:WP - 1],
                             vin[r:r + CHUNK, :, 1:WP]).then_inc(sem_in, 16)
                sp.dma_start(A1S[0].ap()[:, :, 0:WP - 1],
                             vin[r + 1:r + CHUNK + 1, :, 1:WP]) \
                  .then_inc(sem_in, 16)
            loads(0)
            sp.dma_start(BSH.ap()[0:1, :, :], bh[0:1]).then_inc(sem_bsh, 16)
            for n in range(N):
                if n + 1 < N:
                    sp.wait_ge(vsem, vm[("AREL", n)])
                    loads(n + 1)
                sp.wait_ge(vsem, vm[("BB", n)])
                sp.dma_start(BSH.ap()[1:CHUNK, :, :],
                             BBt.ap()[0:CHUNK - 1, :, :]).then_inc(sem_bsh, 16)
                if n + 1 < N:
                    sp.wait_ge(vsem, vm[("VN", n)])
                    sp.dma_start(BSH.ap()[0:1, :, :],
                                 BBt.ap()[CHUNK - 1:CHUNK, :, :]) \
                      .then_inc(sem_bsh, 16)
                sp.wait_ge(vsem, vm[("OT", n)])
                sp.dma_start(out[rows(n):rows(n) + CHUNK],
                             OT[n % 2].ap()[:, :, 0:WO]).then_inc(sem_out, 16)
    return nc


def _get_nc():
    if "nc" not in _NC_CACHE:
        _NC_CACHE["nc"] = _build_nc()
    return _NC_CACHE["nc"]


# ------------------------------------------------------------------ kernel

def kernel(verts, faces, normmap):
    global LAST_PERF
    verts = np.ascontiguousarray(np.asarray(verts), dtype=np.float32)
    faces = np.asarray(faces)
    normmap = np.asarray(normmap)

    if not _is_grid_mesh(verts, faces, normmap):
        return _fallback(verts, faces, normmap)

    in_maps, gp32 = _build_in_maps(verts)

    from concourse.bass_utils import run_bass_kernel_spmd
    nc = _get_nc()
    res = run_bass_kernel_spmd(nc, in_maps, core_ids=list(range(N_CORES)),
                               trace=TRACE)
    LAST_PERF = res

    outp = np.empty((B, GRID, GRID, 3), np.float32)
    for core in range(N_CORES):
        b, j = divmod(core, 4)
        r0 = j * ROWS
        o = res.results[core]["out"]          # [ROWS, 3, WO] fp16
        outp[b, r0:r0 + ROWS] = o.transpose(0, 2, 1).astype(np.float32)
    for b in range(B):
        last = _normalize3(_host_face_row_b(gp32[b], NCELL - 1))
        outp[b, NCELL] = last.T
    return outp.reshape(B, V, 3)


# revision 11
# speedup vs baseline: 2.3511x; 2.3511x over previous
"""Trainium2 Bass kernel for nn_MeshNorms (gnn_message_passing).

The oracle's inputs are a regular 1025x1025 grid mesh: `faces` / `normmap`
are deterministic functions of the grid, so every gather in the reference is
a shifted-window (stencil) read.  The kernel verifies that structure on the
host (cheap numpy check) and runs a streaming stencil kernel on 8 cores:

  sharding: 2 batches x 4 row-slices; each core computes 256 output rows as
  2 chunks of 128 grid rows (partition dim = grid row).

  All device math is fp16 so every DVE tensor_tensor op engages the 2x_1P
  perf mode (16-bit dtype, unit stride, 4B-aligned, even width).  Shifted
  (odd-offset) operands are produced either by extra shifted DMA loads from
  DRAM (a0s/a1s) or by ACT-engine copies (m(j+1), q(j+1)) which run at 1x
  regardless of alignment.  Squares and rsqrt (raw InstActivation(Rsqrt),
  max rel err ~5e-5, bias=eps folded in) run on the ACT engine in parallel
  with the DVE stream.  tri1/tri2 face-normal planes are packed side by side
  ([... , 2*1028]) so one instruction covers both triangle families.

  The DVE instruction stream is software-pipelined: chunk n's tail
  (vertex-sum normalize + output) is emitted after chunk n+1's head so the
  ACT/DMA latencies hide behind useful DVE work.

Boundary handling: vertex columns are replicate-padded on the host (padded
cells produce exact zero normals); the row-1024 output and the per-core
b-halo row are computed on the host (tiny).

If the structure check fails, falls back to a numpy implementation.
"""

import numpy as np

GRID = 1025
NCELL = GRID - 1
V = GRID * GRID
F = 2 * NCELL * NCELL
B = 2
WP = GRID + 2              # 1027 padded vertex cols
WT = 1028                  # per-plane tile width (even)
W2 = 2 * WT                # paired tri1|tri2 width
WE = 1026                  # even valid width for face/sum ops
WO = GRID                  # 1025 output cols
CHUNK = 128
NCHUNK = 2
ROWS = CHUNK * NCHUNK      # 256 output rows per core
N_CORES = 8
EPS = 1e-6

_NC_CACHE = {}
TRACE = False
LAST_PERF = None


# ---------------------------------------------------------------- host math

def _grid_faces(n):
    idx = np.arange(n * n, dtype=np.int64).reshape(n, n)
    v00 = idx[:-1, :-1]; v01 = idx[:-1, 1:]
    v10 = idx[1:, :-1]; v11 = idx[1:, 1:]
    tri1 = np.stack([v00, v10, v01], axis=-1).reshape(-1, 3)
    tri2 = np.stack([v01, v10, v11], axis=-1).reshape(-1, 3)
    return np.concatenate([tri1, tri2], axis=0)


def _expected_normmap(n):
    nc = n - 1
    i, j = np.meshgrid(np.arange(n, dtype=np.int64),
                       np.arange(n, dtype=np.int64), indexing="ij")
    sent = np.int64(1) << 60

    def t1(ii, jj):
        valid = (ii >= 0) & (ii < nc) & (jj >= 0) & (jj < nc)
        return np.where(valid, ii * nc + jj, sent)

    def t2(ii, jj):
        valid = (ii >= 0) & (ii < nc) & (jj >= 0) & (jj < nc)
        return np.where(valid, nc * nc + ii * nc + jj, sent)

    cand = np.stack([t1(i - 1, j), t1(i, j - 1), t1(i, j),
                     t2(i - 1, j - 1), t2(i - 1, j), t2(i, j - 1)], axis=-1)
    cand.sort(axis=-1)
    cand = cand.reshape(n * n, 6)
    cand[cand == sent] = 2 * nc * nc
    return cand


def _is_grid_mesh(verts, faces, normmap):
    if verts.shape != (B, V, 3) or faces.shape != (F, 3) or normmap.shape != (V, 6):
        return False
    if not np.array_equal(faces, _grid_faces(GRID)):
        return False
    return np.array_equal(normmap, _expected_normmap(GRID))


def _fallback(verts, faces, normmap):
    verts = np.asarray(verts, np.float32)
    faces = np.asarray(faces)
    normmap = np.asarray(normmap)
    tri = verts[:, faces, :]
    v1 = tri[..., 0, :] - tri[..., 1, :]
    v2 = tri[..., 0, :] - tri[..., 2, :]
    cr = np.cross(v1, v2).astype(np.float32)
    fn = cr / np.linalg.norm(cr, axis=-1, keepdims=True)
    bb = fn.shape[0]
    fnp = np.concatenate([fn, np.zeros((bb, 1, 3), fn.dtype)], axis=1)
    vn = fnp[:, normmap, :].sum(axis=-2)
    vn = vn / np.linalg.norm(vn, axis=-1, keepdims=True)
    return vn.astype(np.float32)


def _cross3(u, v):
    return np.stack([u[1] * v[2] - u[2] * v[1],
                     u[2] * v[0] - u[0] * v[2],
                     u[0] * v[1] - u[1] * v[0]], 0).astype(np.float32)


def _normalize3(x, eps=np.float32(1e-12)):
    nsq = (x[0] * x[0] + x[1] * x[1]) + x[2] * x[2]
    s = np.sqrt(nsq + eps, dtype=np.float32)
    return (x * (np.float32(1.0) / s)).astype(np.float32)


def _host_face_row_b(gp, fr):
    """b(fr, j) = m(j+1) + p(j) + p(j+1) for one face row from the padded
    planar f32 grid gp [3, GRID, WP].  Returns [3, WO] float32."""
    WF = GRID + 1
    a0 = gp[:, fr:fr + 1, :]
    a1 = gp[:, fr + 1:fr + 2, :]
    er = a0 - a1
    ec = a0[:, :, :WF] - a0[:, :, 1:]
    dd = a0[:, :, 1:] - a1[:, :, :WF]
    m = _normalize3(_cross3(er[:, :, :WF], ec))
    p = _normalize3(_cross3(dd, er[:, :, 1:]))
    u = m[:, :, 1:] + p[:, :, :WO]
    bb = u + p[:, :, 1:]
    return bb[:, 0, :]


def _build_in_maps(verts):
    """Host prep: padded fp16 planar slabs per core + f32 halo rows.
    Returns (in_maps, gp32row_fn) where in_maps[core] = {vin, bh}."""
    verts = np.asarray(verts, np.float32)
    g = verts.reshape(B, GRID, GRID, 3)
    # full f32 padded planar grid (for host halo rows + last row)
    gp32 = np.empty((B, 3, GRID, WP), np.float32)
    gp32[:, :, :, 1:GRID + 1] = g.transpose(0, 3, 1, 2)
    gp32[:, :, :, 0] = gp32[:, :, :, 1]
    gp32[:, :, :, GRID + 1] = gp32[:, :, :, GRID]
    gp16 = gp32.astype(np.float16)

    in_maps = []
    for core in range(N_CORES):
        b, j = divmod(core, 4)
        r0 = j * ROWS
        slab = np.ascontiguousarray(
            gp16[b, :, r0:r0 + ROWS + 1, :].transpose(1, 0, 2))
        bh = np.zeros((1, 3, WE), np.float16)
        if j > 0:
            bh[0, :, :WO] = _host_face_row_b(gp32[b], r0 - 1).astype(np.float16)
        in_maps.append({"vin": slab, "bh": bh})
    return in_maps, gp32


# ------------------------------------------------------------- device build

def _act_rsqrt(nc, act, mybir, out, in_, bias):
    """Raw InstActivation(Rsqrt): out = rsqrt(in_ + bias). ~5e-5 max rel."""
    AF = mybir.ActivationFunctionType
    ins = [act.lower_ap(in_),
           mybir.ImmediateValue(dtype=mybir.dt.float32, value=float(bias)),
           mybir.ImmediateValue(dtype=mybir.dt.float32, value=1.0),
           mybir.ImmediateValue(dtype=mybir.dt.float32, value=0.0)]
    return act.add_instruction(mybir.InstActivation(
        name=nc.get_next_instruction_name(), func=AF.Rsqrt,
        ins=ins, outs=[act.lower_ap(out)]))


def _build_nc(repeat=1):
    from contextlib import ExitStack
    import concourse.bass as bass
    import concourse.mybir as mybir

    f16 = mybir.dt.float16
    AF = mybir.ActivationFunctionType

    nc = bass.Bass()
    vin = nc.dram_tensor("vin", [ROWS + 1, 3, WP], f16, kind="ExternalInput")
    bh = nc.dram_tensor("bh", [1, 3, WE], f16, kind="ExternalInput")
    out = nc.dram_tensor("out", [ROWS, 3, WO], f16, kind="ExternalOutput")

    N = NCHUNK * repeat

    # ---- precompute semaphore mark values by replaying emission order ----
    # DVE stream per n: head(n) | VN(n-1) | QB(n) | VQB,OT(n-1) | rest of
    # mid(n); final tail after the loop.
    vm = {}
    cnt = 0
    def _vrec(kind, n):
        nonlocal cnt
        cnt += 1
        vm[(kind, n)] = cnt
    for n in range(N):
        _vrec("ER", n); _vrec("AREL", n); _vrec("NN", n)      # head(n)
        if n > 0:
            _vrec("VN", n - 1)
        _vrec("QB", n)
        if n > 0:
            _vrec("VQB", n - 1); _vrec("OT", n - 1)
        _vrec("MM", n); _vrec("QQ", n); _vrec("BB", n)
    _vrec("VN", N - 1); _vrec("VQB", N - 1); _vrec("OT", N - 1)

    # ACT stream per n: ERS(n) SQ(n) | VSQ(n-1) VRS(n-1) | RS(n) MS(n) QS(n)
    am = {}
    acnt = 0
    def _arec(kind, n):
        nonlocal acnt
        acnt += 1
        am[(kind, n)] = acnt
    for n in range(N):
        _arec("ERS", n); _arec("SQ", n)
        if n > 0:
            _arec("VSQ", n - 1); _arec("VRS", n - 1)
        _arec("RS", n); _arec("MS", n); _arec("QS", n)
    _arec("VSQ", N - 1); _arec("VRS", N - 1)

    with ExitStack() as ctx:
        sb = lambda shape, name: ctx.enter_context(
            nc.sbuf_tensor(name, shape, f16))
        A0 = [sb([CHUNK, 3, WT], "a0_0")]
        A1 = [sb([CHUNK, 3, WT], "a1_0")]
        A0S = [sb([CHUNK, 3, WT], "a0s_0")]
        E = sb([CHUNK, 3, W2], "E")      # er | dd
        Ft = sb([CHUNK, 3, W2], "F")     # ec | ers
        T1 = sb([CHUNK, 3, W2], "T1")
        T2 = sb([CHUNK, 3, W2], "T2")
        NNt = sb([CHUNK, 3, W2], "NN")   # n1 | n2
        QBt = sb([CHUNK, W2], "QB")
        RRt = sb([CHUNK, W2], "RR")
        MMt = sb([CHUNK, 3, W2], "MM")   # m | p
        MS = sb([CHUNK, 3, WE], "MS")    # m(j+1)
        QS = sb([CHUNK, 3, WE], "QS")    # q(j+1)
        QQ = sb([CHUNK, 3, WE], "QQ")    # m + p
        AA = sb([CHUNK, 3, WE], "AA")
        VN = sb([CHUNK, 3, WE], "VN")
        BBt = sb([CHUNK, 3, WE], "BB")
        BSH = sb([CHUNK, 3, WE], "BSH")
        VQB = sb([CHUNK, WE], "VQB")
        RV = sb([CHUNK, WE], "RV")
        OT = [sb([CHUNK, 3, WE], f"ot_{s}") for s in range(2)]

        # aliases into dead tiles (guarded by asem waits in head())
        SQ = T1            # ACT squares of NN        [3, W2]
        VSQ = T2           # ACT vertex squares ->    [3, 0:WE]

        sem_in = ctx.enter_context(nc.semaphore("sem_in"))
        sem_bsh = ctx.enter_context(nc.semaphore("sem_bsh"))
        sem_out = ctx.enter_context(nc.semaphore("sem_out"))
        vsem = ctx.enter_context(nc.semaphore("vsem"))
        asem = ctx.enter_context(nc.semaphore("asem"))
        block = ctx.enter_context(nc.Block())

        def rows(n):
            r = (n % NCHUNK) * CHUNK
            return r

        @block.vector
        def _(dve):
            def seg2(ap3, c, width=WE):
                """[128, 2, width] view pairing the tri1/tri2 segments of
                component c of a [CHUNK, 3, W2] tile."""
                return ap3[:, c, 0:W2].rearrange(
                    "p (s w) -> p s w", s=2)[:, :, 0:width]

            def head(n):
                a0, a1, a0s = A0[0].ap(), A1[0].ap(), A0S[0].ap()
                e, f = E.ap(), Ft.ap()
                dve.wait_ge(sem_in, 48 * (n + 1))
                dve.tensor_sub(e[:, :, 0:WT], a0, a1).then_inc(vsem, 1)  # ER
                dve.tensor_sub(e[:, :, WT:W2], a0s, a1)                  # dd
                dve.tensor_sub(f[:, :, 0:WT], a0, a0s).then_inc(vsem, 1)  # AREL
                # T1/T2 are also ACT's SQ(n-1)/VSQ(n-2) outputs and NNt is
                # SQ(n-1)'s input — guard the aliases; ers copy is ACT's.
                if n >= 1:
                    dve.wait_ge(asem, am[("SQ", n - 1)])
                if n >= 2:
                    dve.wait_ge(asem, am[("VSQ", n - 2)])
                dve.wait_ge(asem, am[("ERS", n)])
                t1, t2, nn = T1.ap(), T2.ap(), NNt.ap()
                for c in range(3):
                    u1, u2 = (c + 1) % 3, (c + 2) % 3
                    dve.tensor_mul(seg2(t1, c), seg2(e, u1), seg2(f, u2))
                for c in range(3):
                    u1, u2 = (c + 1) % 3, (c + 2) % 3
                    dve.tensor_mul(seg2(t2, c), seg2(e, u2), seg2(f, u1))
                dve.tensor_sub(nn, t1, t2).then_inc(vsem, 1)             # NN

            def tail_vn(n):
                dve.wait_ge(sem_bsh, 32 * n + 32)
                dve.tensor_add(VN.ap(), AA.ap(), BSH.ap()) \
                   .then_inc(vsem, 1)                                    # VN

            def mid_qb(n):
                qb, sq = QBt.ap(), SQ.ap()
                dve.wait_ge(asem, am[("SQ", n)])
                dve.tensor_add(qb, sq[:, 0, :], sq[:, 1, :])
                dve.tensor_add(qb, qb, sq[:, 2, :]).then_inc(vsem, 1)    # QB

            def tail_out(n):
                vsq = VSQ.ap()
                dve.wait_ge(asem, am[("VSQ", n)])
                dve.tensor_add(VQB.ap(), vsq[:, 0, 0:WE], vsq[:, 1, 0:WE])
                dve.tensor_add(VQB.ap(), VQB.ap(),
                               vsq[:, 2, 0:WE]).then_inc(vsem, 1)        # VQB
                dve.wait_ge(asem, am[("VRS", n)])
                if n >= 2:
                    dve.wait_ge(sem_out, 16 * (n - 1))
                ot = OT[n % 2].ap()
                ins = None
                for c in range(3):
                    ins = dve.tensor_mul(ot[:, c, :], VN.ap()[:, c, :], RV.ap())
                ins.then_inc(vsem, 1)                                    # OT

            def mid_rest(n):
                rr, nn, mm = RRt.ap(), NNt.ap(), MMt.ap()
                dve.wait_ge(asem, am[("RS", n)])
                ins = None
                for c in range(3):
                    ins = dve.tensor_mul(mm[:, c, :], nn[:, c, :], rr)
                ins.then_inc(vsem, 1)                                    # MM
                dve.tensor_add(QQ.ap(), mm[:, :, 0:WE],
                               mm[:, :, WT:WT + WE]).then_inc(vsem, 1)   # QQ
                dve.wait_ge(asem, am[("MS", n)])
                dve.tensor_add(AA.ap(), QQ.ap(), MS.ap())
                dve.wait_ge(asem, am[("QS", n)])
                if n >= 1:
                    dve.wait_ge(sem_bsh, 32 * n + 16)   # bsh0(n) landed
                dve.tensor_add(BBt.ap(), QS.ap(),
                               mm[:, :, WT:WT + WE]).then_inc(vsem, 1)   # BB

            for n in range(N):
                head(n)
                if n > 0:
                    tail_vn(n - 1)
                mid_qb(n)
                if n > 0:
                    tail_out(n - 1)
                mid_rest(n)
            tail_vn(N - 1)
            tail_out(N - 1)

        @block.scalar
        def _(act):
            def vtail(n):
                act.wait_ge(vsem, vm[("VN", n)])
                ins = None
                for c in range(3):
                    ins = act.activation(VSQ.ap()[:, c, 0:WE],
                                         VN.ap()[:, c, :], AF.Square)
                ins.then_inc(asem, 1)                                    # VSQ
                act.wait_ge(vsem, vm[("VQB", n)])
                _act_rsqrt(nc, act, mybir, RV.ap(), VQB.ap(), EPS) \
                    .then_inc(asem, 1)                                   # VRS
            for n in range(N):
                act.wait_ge(vsem, vm[("ER", n)])
                act.activation(Ft.ap()[:, :, WT:WT + WE],
                               E.ap()[:, :, 1:1 + WE], AF.Copy) \
                    .then_inc(asem, 1)                                   # ERS
                act.wait_ge(vsem, vm[("NN", n)])
                ins = None
                for c in range(3):
                    ins = act.activation(SQ.ap()[:, c, :], NNt.ap()[:, c, :],
                                         AF.Square)
                ins.then_inc(asem, 1)                                    # SQ
                if n > 0:
                    vtail(n - 1)
                act.wait_ge(vsem, vm[("QB", n)])
                _act_rsqrt(nc, act, mybir, RRt.ap(), QBt.ap(), EPS) \
                    .then_inc(asem, 1)                                   # RS
                act.wait_ge(vsem, vm[("MM", n)])
                act.activation(MS.ap(), MMt.ap()[:, :, 1:1 + WE], AF.Copy) \
                    .then_inc(asem, 1)                                   # MS
                act.wait_ge(vsem, vm[("QQ", n)])
                act.activation(QS.ap()[:, :, 0:WE - 1],
                               QQ.ap()[:, :, 1:WE], AF.Copy) \
                    .then_inc(asem, 1)                                   # QS
            vtail(N - 1)

        @block.sync
        def _(sp):
            def loads(n):
                r = rows(n)
                sp.dma_start(A0[0].ap()[:, :, 0:WP], vin[r:r + CHUNK]) \
                  .then_inc(sem_in, 16)
                sp.dma_start(A1[0].ap()[:, :, 0:WP], vin[r + 1:r + CHUNK + 1]) \
                  .then_inc(sem_in, 16)
                sp.dma_start(A0S[0].ap()[:, :, 0:WP - 1],
                             vin[r:r + CHUNK, :, 1:WP]).then_inc(sem_in, 16)
            loads(0)
            sp.dma_start(BSH.ap()[0:1, :, :], bh[0:1]).then_inc(sem_bsh, 16)
            for n in range(N):
                if n + 1 < N:
                    sp.wait_ge(vsem, vm[("AREL", n)])
                    loads(n + 1)
                sp.wait_ge(vsem, vm[("BB", n)])
                sp.dma_start(BSH.ap()[1:CHUNK, :, :],
                             BBt.ap()[0:CHUNK - 1, :, :]).then_inc(sem_bsh, 16)
                if n + 1 < N:
                    sp.wait_ge(vsem, vm[("VN", n)])
                    sp.dma_start(BSH.ap()[0:1, :, :],
                                 BBt.ap()[CHUNK - 1:CHUNK, :, :]) \
                      .then_inc(sem_bsh, 16)
                sp.wait_ge(vsem, vm[("OT", n)])
                sp.dma_start(out[rows(n):rows(n) + CHUNK],
                             OT[n % 2].ap()[:, :, 0:WO]).then_inc(sem_out, 16)
    return nc


def _get_nc():
    if "nc" not in _NC_CACHE:
        _NC_CACHE["nc"] = _build_nc()
    return _NC_CACHE["nc"]


# ------------------------------------------------------------------ kernel

def kernel(verts, faces, normmap):
    global LAST_PERF
    verts = np.ascontiguousarray(np.asarray(verts), dtype=np.float32)
    faces = np.asarray(faces)
    normmap = np.asarray(normmap)

    if not _is_grid_mesh(verts, faces, normmap):
        return _fallback(verts, faces, normmap)

    in_maps, gp32 = _build_in_maps(verts)

    from concourse.bass_utils import run_bass_kernel_spmd
    nc = _get_nc()
    res = run_bass_kernel_spmd(nc, in_maps, core_ids=list(range(N_CORES)),
                               trace=TRACE)
    LAST_PERF = res

    outp = np.empty((B, GRID, GRID, 3), np.float32)
    for core in range(N_CORES):
        b, j = divmod(core, 4)
        r0 = j * ROWS
        o = res.results[core]["out"]          # [ROWS, 3, WO] fp16
        outp[b, r0:r0 + ROWS] = o.transpose(0, 2, 1).astype(np.float32)
    for b in range(B):
        last = _normalize3(_host_face_row_b(gp32[b], NCELL - 1))
        outp[b, NCELL] = last.T
    return outp.reshape(B, V, 3)
